# revision 41
# baseline (speedup 1.0000x reference)
"""Trainium2 Bass kernel: AdaptiveGroupCorrelationLayer.

Sharding: 8 cores = 4 batches x 2 channel-halves (128 ch each = 2 groups of 64).
Per core:
  1. Load left/right [128c, 96, 128] f32 -> bf16 (cast-DMA), flow/grid [96,128].
  2. Param math on [96h, 128w]: bilinear coords, 4 weights (1/64 folded),
     4 gather indices (invalid neighbors -> zero-token 12288).
  3. right -> token layout [w, h, c] via dma_start_transpose; zero pad stripe.
  4. 4x dma_gather (SBUF-source, transpose mode) -> G_n [128c, pix] bf16.
  5. Weights broadcast [1,pix] -> [128,pix] via PE outer product + ACT copy.
  6. Lerp: R_w = sum w_n*G_n (7 DVE TT ops), written edge-padded (even+odd).
  7. Taps: T_k = L * shift_k(R_w); masked-selector matmuls accumulate
     corr[(g,k), pix] in PSUM [18, 512] chunks over the 9 taps.
  8. ACT copy -> bf16, cast-DMA out to [18, 96, 128] f32.
"""
import sys, os
sys.path.insert(0, '/opt/trn_rl_repo')
import numpy as np
KDBG = bool(os.environ.get('KDBG'))
KSTAGE = os.environ.get('KSTAGE', 'full')

import concourse.bass as bass
from concourse import bacc
import concourse.mybir as mybir
from concourse.tile import TileContext
from concourse.bass_utils import run_bass_kernel_spmd

F32 = mybir.dt.float32
BF16 = mybir.dt.bfloat16
I16 = mybir.dt.int16
ALU = mybir.AluOpType

H, W, C = 96, 128, 128       # per-core shard
NPIX = H * W                 # 12288
NCH = 6                      # h-chunks
CROWS = H // NCH             # 16 rows per chunk
CPIX = CROWS * W             # 2048 px per chunk
ZTOK = NPIX                  # zero token index
SUB = 512


def build_nc():
    nc = bacc.Bacc()
    left_e = nc.declare_dram_parameter("left", [C, H, W], F32, isOutput=False)
    right_e = nc.declare_dram_parameter("right", [C, H, W], F32, isOutput=False)
    flow_e = nc.declare_dram_parameter("flow", [2, H, W], F32, isOutput=False)
    grid_e = nc.declare_dram_parameter("grid", [2, H, W], F32, isOutput=False)
    ones_e = nc.declare_dram_parameter("ones1", [1, 128], F32, isOutput=False)
    selw_e = nc.declare_dram_parameter("selw", [9, 128, 18], F32, isOutput=False)
    out_e = nc.declare_dram_parameter("out", [18, H, W], F32, isOutput=True)

    wscr = nc.dram_tensor("wscratch", [H, 4, W], BF16)
    iscr = nc.dram_tensor("iscratch", [H, 2, W], I16)
    if KDBG:
        dbg_warp = nc.declare_dram_parameter("dbg_warp", [C, H, W], F32, isOutput=True)
        dbg_idx = nc.declare_dram_parameter("dbg_idx", [128, 4, NPIX // 16], I16, isOutput=True)
        dbg_g = nc.declare_dram_parameter("dbg_g", [C, 4, CPIX], F32, isOutput=True)
        dbg_w = nc.declare_dram_parameter("dbg_w", [C, 4, CPIX], F32, isOutput=True)

    with TileContext(nc) as tc:
        with (
            tc.tile_pool(name="per", bufs=1) as per,
            tc.tile_pool(name="rbp", bufs=1) as rbp,
            tc.tile_pool(name="gp", bufs=2) as gp,
            tc.tile_pool(name="wp", bufs=1) as wp,
            tc.tile_pool(name="wfp", bufs=1) as wfp,
            tc.tile_pool(name="lp", bufs=2) as lp,
            tc.tile_pool(name="ltp", bufs=1) as ltp,
            tc.tile_pool(name="tkp", bufs=3) as tkp,
            tc.tile_pool(name="osp", bufs=2) as osp,
            tc.tile_pool(name="cps", bufs=4, space="PSUM") as cps,
            tc.tile_pool(name="wps", bufs=2, space="PSUM") as wps,
        ):
            # ---------- persistent loads ----------
            L = per.tile([C, NPIX], BF16, tag="L", name="L")
            nc.gpsimd.dma_start(L[:], left_e[:].rearrange("c h w -> c (h w)"))

            ones_tile = per.tile([65, 128], BF16, tag="ones", name="ones")
            nc.gpsimd.dma_start(ones_tile[0:1, :], ones_e[:])
            nc.gpsimd.dma_start(ones_tile[64:65, :], ones_e[:])
            ones_sb = ones_tile[0:1, :]
            ones64 = ones_tile[64:65, :]
            selw_sb = per.tile([128, 9, 18], BF16, tag="selw", name="selw")
            nc.gpsimd.dma_start(
                selw_sb[:], selw_e[:].rearrange("k c j -> c k j"))

            # ---------- tokens: [w-part, rank h, c] + zero stripe ----------
            # rank r holds [row r-1 | row r] (512B); rows duplicated across slots
            tok = per.tile([128, H + 3, 2, C], BF16, tag="tok", name="tok")
            nc.gpsimd.memset(tok[:, 0, 0, :], 0)
            nc.gpsimd.memset(tok[:, H, 1, :], 0)
            nc.gpsimd.memset(tok[:, H + 1, :, :], 0)
            nc.gpsimd.memset(tok[:, H + 2, :, :], 0)
            for rh in ([] if KSTAGE == 'tok0' else [0, 1, 2, 3]):
                rbh = rbp.tile([C, NPIX // 4], BF16, tag="rbh", name="rbh")
                nc.gpsimd.dma_start(
                    rbh[:],
                    right_e[:, rh * (H // 4):(rh + 1) * (H // 4), :]
                    .rearrange("c h w -> c (h w)"))
                nc.sync.dma_start_transpose(
                    tok[:, 1 + rh * (H // 4):1 + (rh + 1) * (H // 4), 0, :], rbh[:])
                nc.sync.dma_start_transpose(
                    tok[:, rh * (H // 4):(rh + 1) * (H // 4), 1, :], rbh[:])

            # ---------- params on [96h, 128w], all in one sliced tile -------
            pt = per.tile([H, 25, W], F32, tag="pt", name="pt")
            slot = {}

            def P(tag):
                if tag not in slot:
                    slot[tag] = len(slot)
                    assert len(slot) <= 25, slot
                return pt[:, slot[tag], :]

            def TT(dst, a, b, op):
                nc.vector.tensor_tensor(out=dst, in0=a, in1=b, op=op)

            def TS(dst, a, s1, s2, op0, op1=None):
                kw = {} if op1 is None else {"op1": op1}
                nc.vector.tensor_scalar(out=dst, in0=a, scalar1=s1, scalar2=s2,
                                        op0=op0, **kw)

            nc.sync.dma_start(P("fxin"), flow_e[0])
            nc.sync.dma_start(P("fyin"), flow_e[1])
            nc.sync.dma_start(P("gx"), grid_e[0])
            nc.sync.dma_start(P("gy"), grid_e[1])

            TT(P("x"), P("gx"), P("fxin"), ALU.add)
            TT(P("y"), P("gy"), P("fyin"), ALU.add)
            MAGIC = 12582912.0  # 1.5*2**23 (ulp=1): floor via round(x-0.5)
            TS(P("x0"), P("x"), -0.5, MAGIC, ALU.add, ALU.add)
            TS(P("x0"), P("x0"), -MAGIC, None, ALU.add)
            TS(P("y0"), P("y"), -0.5, MAGIC, ALU.add, ALU.add)
            TS(P("y0"), P("y0"), -MAGIC, None, ALU.add)
            TT(P("fx"), P("x"), P("x0"), ALU.subtract)
            TT(P("fy"), P("y"), P("y0"), ALU.subtract)

            def valid(src, lo, hi, tag):
                TS(P(tag + "a"), src, float(lo), None, ALU.is_ge)
                TS(P("vtmp"), src, float(hi), None, ALU.is_le)
                TT(P(tag + "a"), P(tag + "a"), P("vtmp"), ALU.mult)
                return P(tag + "a")

            vx0 = valid(P("x0"), 0, 127, "vx0")
            vx1 = valid(P("x0"), -1, 126, "vx1")
            vy0 = valid(P("y0"), 0, 95, "vy0")
            vy1 = valid(P("y0"), -1, 94, "vy1")

            s = 1.0 / 64.0
            TS(P("wx0"), P("fx"), -s, s, ALU.mult, ALU.add)
            TS(P("wx1"), P("fx"), s, None, ALU.mult)
            TS(P("wy0"), P("fy"), -1.0, 1.0, ALU.mult, ALU.add)
            TS(P("wy1"), P("fy"), 1.0, None, ALU.mult)

            wmap = per.tile([H, 4, W], F32, tag="wmap", name="wmap")
            TT(wmap[:, 0, :], P("wx0"), P("wy0"), ALU.mult)
            TT(wmap[:, 1, :], P("wx0"), P("wy1"), ALU.mult)
            TT(wmap[:, 2, :], P("wx1"), P("wy0"), ALU.mult)
            TT(wmap[:, 3, :], P("wx1"), P("wy1"), ALU.mult)

            TS(P("xc0"), P("x0"), 0.0, 127.0, ALU.max, ALU.min)
            TS(P("xc1"), P("x0"), 1.0, 0.0, ALU.add, ALU.max)
            TS(P("xc1"), P("xc1"), 127.0, None, ALU.min)
            # pair row base: (y0+1)*128, pair-validity y0 in [-1, 95]
            TS(P("yb0"), P("y0"), 1.0, 128.0, ALU.add, ALU.mult)
            TS(P("pya"), P("y0"), -1.0, None, ALU.is_ge)
            TS(P("vtmp"), P("y0"), 95.0, None, ALU.is_le)
            TT(P("pya"), P("pya"), P("vtmp"), ALU.mult)

            ZP = 97 * 128  # zero-pair token (ranks 97,98 are zero)
            imap = per.tile([H, 2, W], F32, tag="imap", name="imap")
            for n, (xcn, vxn) in enumerate([("xc0", vx0), ("xc1", vx1)]):
                TT(P("bn"), P("yb0"), P(xcn), ALU.add)
                TT(P("vn"), P("pya"), vxn, ALU.mult)
                TS(P("bn"), P("bn"), -float(ZP), None, ALU.add)
                TT(P("bn"), P("bn"), P("vn"), ALU.mult)
                TS(imap[:, n, :], P("bn"), float(ZP), None, ALU.add)

            # cast + micro-permute w -> (q*8 + wh) in one copy
            imap16t = per.tile([H, 2, W], I16, tag="imap16t", name="imap16t")
            nc.vector.tensor_copy(
                imap16t[:].rearrange("h n (q wh) -> h n q wh", q=16),
                imap[:].rearrange("h n (wh q) -> h n q wh", wh=8))

            # weights/indices via DRAM rearrange bounce
            nc.gpsimd.dma_start(wscr[:], wmap[:])
            nc.sync.dma_start(iscr[:], imap16t[:])

            idx_all = per.tile([128, 2, NPIX // 16], I16, tag="idx", name="idx")
            # row p=(ph*16+q) holds, at (n, j), idx of pixel j*16 + q
            # j = h*8 + wh ; pixel = h*128 + wh*16 + q
            # iscr[h, n, q*8 + wh] = idx_n(h, wh*16 + q)
            for n in range(2):
                nc.sync.dma_start(
                    idx_all[0:16, n].rearrange("q (h wh) -> q h wh", wh=8),
                    iscr[:, n].rearrange("h (q wh) -> q h wh", q=16))
            for ph in range(1, 8):
                nc.sync.dma_start(idx_all[ph * 16:(ph + 1) * 16], idx_all[0:16])

            # ---------- per-chunk pipeline ----------
            for ci in range(NCH):
                p0 = ci * CPIX
                h0 = ci * CROWS
                # gathers
                G4 = gp.tile([128, 4, CPIX], BF16, tag="G", name="G%d" % ci)
                for n in range(2 if KSTAGE != 'noga' else 0):
                    nc.gpsimd.dma_gather(
                        out_ap=G4[:, 2 * n:2 * n + 2, :],
                        in_ap=tok[:].rearrange("p h s c -> p (h s c)"),
                        idxs_ap=idx_all[:, n, ci * (CPIX // 16):(ci + 1) * (CPIX // 16)],
                        num_idxs=CPIX, num_idxs_reg=CPIX, elem_size=2 * C,
                        transpose=True, sbuf_tokens_per_rank=128,
                        sbuf_free_dim_per_rank=C * 4, single_packet=False)

                if KSTAGE in ('tok', 'idx', 'noga'):
                    osbg2 = osp.tile([18, CPIX], BF16, tag="osb", name="osbg2%d" % ci)
                    if KSTAGE in ('tok', 'noga'):
                        nc.vector.tensor_copy(
                            osbg2[:], tok[0:18, h0:h0 + CROWS, :]
                            .rearrange("p h c -> p (h c)"))
                    else:
                        nc.vector.tensor_copy(
                            osbg2[:, 0:768],
                            idx_all[0:18, 0, :].bitcast(BF16))
                    nc.gpsimd.dma_start(
                        out_e[:, h0:h0 + CROWS, :].rearrange("j h w -> j (h w)"),
                        osbg2[:])
                    continue
                if KSTAGE == 'gather':
                    osbg = osp.tile([18, CPIX], BF16, tag="osb", name="osbg%d" % ci)
                    nc.vector.tensor_copy(osbg[:], G4[0:18, 0, :])
                    nc.gpsimd.dma_start(
                        out_e[:, h0:h0 + CROWS, :].rearrange("j h w -> j (h w)"),
                        osbg[:])
                    continue
                # weight maps for this chunk: [1, 4*CPIX] bf16 <- DRAM
                wfc = wfp.tile([65, 2, CPIX], BF16, tag="wfc", name="wfc%d" % ci)
                for n in range(4):
                    nc.sync.dma_start(
                        wfc[(n % 2) * 64:(n % 2) * 64 + 1, n // 2]
                        .rearrange("o (h w) -> o h w", w=W),
                        wscr[h0:h0 + CROWS, n])
                W4 = wp.tile([128, 4, CPIX], BF16, tag="W", name="W%d" % ci)
                for n in range(4):
                    for hf in range(2):
                        ps = wps.tile([128, CPIX // 2], F32, tag="wps",
                                      name="wps")
                        for sb in range(2):
                            o0 = hf * (CPIX // 2) + sb * SUB
                            nc.tensor.matmul(
                                ps[:, sb * SUB:(sb + 1) * SUB],
                                ones_sb if n % 2 == 0 else ones64,
                                wfc[(n % 2) * 64:(n % 2) * 64 + 1, n // 2,
                                    o0:o0 + SUB])
                        nc.scalar.copy(
                            W4[:, n, hf * (CPIX // 2):(hf + 1) * (CPIX // 2)],
                            ps[:])

                # lerp -> padded tiles
                pe = lp.tile([128, CROWS, 136], BF16, tag="pe", name="pe%d" % ci)
                po = lp.tile([128, CROWS, 138], BF16, tag="po", name="po%d" % ci)
                t0 = ltp.tile([128, CPIX], BF16, tag="lt0", name="lt0")
                t1 = ltp.tile([128, CPIX], BF16, tag="lt1", name="lt1")
                TT(t0[:], G4[:, 0, :], W4[:, 0, :], ALU.mult)
                TT(t1[:], G4[:, 1, :], W4[:, 1, :], ALU.mult)
                TT(t0[:], t0[:], t1[:], ALU.add)
                TT(t1[:], G4[:, 2, :], W4[:, 2, :], ALU.mult)
                TT(t0[:], t0[:], t1[:], ALU.add)
                TT(t1[:], G4[:, 3, :], W4[:, 3, :], ALU.mult)
                nc.vector.tensor_tensor(
                    out=pe[:, :, 4:132],
                    in0=t0[:].rearrange("c (h w) -> c h w", w=W),
                    in1=t1[:].rearrange("c (h w) -> c h w", w=W), op=ALU.add)
                for j in range(4):
                    nc.vector.tensor_copy(pe[:, :, j:j + 1], pe[:, :, 4:5])
                    nc.vector.tensor_copy(pe[:, :, 132 + j:133 + j],
                                          pe[:, :, 131:132])
                nc.sync.dma_start(po[:, :, 1:137], pe[:, :, 0:136])
                if KDBG:
                    nc.gpsimd.dma_start(
                        dbg_warp[:, h0:h0 + CROWS, :], pe[:, :, 4:132])
                    if ci == 0:
                        nc.gpsimd.dma_start(dbg_idx[:], idx_all[:])
                        nc.gpsimd.dma_start(dbg_g[:], G4[:])
                        nc.gpsimd.dma_start(dbg_w[:], W4[:])

                # taps + selector matmuls
                osb = osp.tile([18, CPIX], F32, tag="osb", name="osb%d" % ci)
                if KSTAGE == 'warp':
                    nc.vector.tensor_copy(
                        osb[:].rearrange("c (h w) -> c h w", w=W),
                        pe[0:18, :, 4:132])
                    nc.gpsimd.dma_start(
                        out_e[:, h0:h0 + CROWS, :].rearrange("j h w -> j (h w)"),
                        osb[:])
                    continue
                for tc2 in range(2):
                    q0 = tc2 * (CPIX // 2)
                    lsl = L[:, p0 + q0: p0 + q0 + CPIX // 2]
                    pss = [cps.tile([128, SUB], F32, tag="cpsum",
                                    name="cps%d_%d_%d" % (ci, tc2, i))
                           for i in range(2)]
                    korder = [0, 2, 4, 6, 8, 1, 3, 5, 7]
                    for ki, k in enumerate(korder):
                        tk = tkp.tile([128, CPIX // 2], BF16, tag="tk",
                                      name="tk")
                        if k % 2 == 0:
                            rw = pe[:, tc2 * 8:(tc2 + 1) * 8, k:k + 128]
                        else:
                            rw = po[:, tc2 * 8:(tc2 + 1) * 8, k + 1:k + 129]
                        nc.vector.tensor_tensor(
                            out=tk[:].rearrange("c (h w) -> c h w", w=W),
                            in0=lsl.rearrange("c (h w) -> c h w", w=W),
                            in1=rw, op=ALU.mult)
                        for sb in range(2):
                            nc.tensor.matmul(
                                pss[sb][0:18, :], selw_sb[:, k, :],
                                tk[:, sb * SUB:(sb + 1) * SUB],
                                start=(ki == 0), stop=(ki == 8))
                    for sb in range(2):
                        nc.scalar.copy(
                            osb[:, q0 + sb * SUB:q0 + (sb + 1) * SUB],
                            pss[sb][0:18, :])
                nc.scalar.dma_start(
                    out_e[:, h0:h0 + CROWS, :].rearrange("j h w -> j (h w)"),
                    osb[:])
    nc.compile()
    return nc


_NC_CACHE = {}


def _get_nc():
    if "nc" not in _NC_CACHE:
        _NC_CACHE["nc"] = build_nc()
    return _NC_CACHE["nc"]


def _consts():
    yy, xx = np.meshgrid(np.arange(H, dtype=np.float32),
                         np.arange(W, dtype=np.float32), indexing="ij")
    grid = np.stack([xx, yy], axis=0).astype(np.float32)  # [2, H, W] (x, y)
    ones1 = np.ones((1, 128), np.float32)
    selw = np.zeros((9, 128, 18), np.float32)
    for k in range(9):
        for c in range(128):
            g = c // 64
            selw[k, c, g * 9 + k] = 1.0
    return grid, ones1, selw


def _in_maps(left_features, right_features, flow):
    grid, ones1, selw = _consts()
    in_maps = []
    for core in range(8):
        b, half = core // 2, core % 2
        in_maps.append({
            "left": np.ascontiguousarray(
                left_features[b, half * 128:(half + 1) * 128]).astype(np.float32),
            "right": np.ascontiguousarray(
                right_features[b, half * 128:(half + 1) * 128]).astype(np.float32),
            "flow": np.ascontiguousarray(flow[b]).astype(np.float32),
            "grid": grid, "ones1": ones1, "selw": selw,
        })
    return in_maps


def kernel(left_features, right_features, flow):
    nc = _get_nc()
    res = run_bass_kernel_spmd(nc, _in_maps(left_features, right_features, flow),
                               core_ids=list(range(8)))
    out = np.zeros((4, 36, H, W), np.float32)
    for core in range(8):
        b, half = core // 2, core % 2
        out[b, half * 18:(half + 1) * 18] = res.results[core]["out"]
    return out


# revision 42
# speedup vs baseline: 1.0006x; 1.0006x over previous
"""Trainium2 Bass kernel: AdaptiveGroupCorrelationLayer.

Sharding: 8 cores = 4 batches x 2 channel-halves (128 ch each = 2 groups of 64).
Per core:
  1. Load left/right [128c, 96, 128] f32 -> bf16 (cast-DMA), flow/grid [96,128].
  2. Param math on [96h, 128w]: bilinear coords, 4 weights (1/64 folded),
     4 gather indices (invalid neighbors -> zero-token 12288).
  3. right -> token layout [w, h, c] via dma_start_transpose; zero pad stripe.
  4. 4x dma_gather (SBUF-source, transpose mode) -> G_n [128c, pix] bf16.
  5. Weights broadcast [1,pix] -> [128,pix] via PE outer product + ACT copy.
  6. Lerp: R_w = sum w_n*G_n (7 DVE TT ops), written edge-padded (even+odd).
  7. Taps: T_k = L * shift_k(R_w); masked-selector matmuls accumulate
     corr[(g,k), pix] in PSUM [18, 512] chunks over the 9 taps.
  8. ACT copy -> bf16, cast-DMA out to [18, 96, 128] f32.
"""
import sys, os
sys.path.insert(0, '/opt/trn_rl_repo')
import numpy as np
KDBG = bool(os.environ.get('KDBG'))
KSTAGE = os.environ.get('KSTAGE', 'full')

import concourse.bass as bass
from concourse import bacc
import concourse.mybir as mybir
from concourse.tile import TileContext
from concourse.bass_utils import run_bass_kernel_spmd

F32 = mybir.dt.float32
BF16 = mybir.dt.bfloat16
I16 = mybir.dt.int16
ALU = mybir.AluOpType

H, W, C = 96, 128, 128       # per-core shard
NPIX = H * W                 # 12288
NCH = 6                      # h-chunks
CROWS = H // NCH             # 16 rows per chunk
CPIX = CROWS * W             # 2048 px per chunk
ZTOK = NPIX                  # zero token index
SUB = 512


def build_nc():
    nc = bacc.Bacc()
    left_e = nc.declare_dram_parameter("left", [C, H, W], F32, isOutput=False)
    right_e = nc.declare_dram_parameter("right", [C, H, W], F32, isOutput=False)
    flow_e = nc.declare_dram_parameter("flow", [2, H, W], F32, isOutput=False)
    grid_e = nc.declare_dram_parameter("grid", [2, H, W], F32, isOutput=False)
    ones_e = nc.declare_dram_parameter("ones1", [1, 128], F32, isOutput=False)
    selw_e = nc.declare_dram_parameter("selw", [9, 128, 18], F32, isOutput=False)
    out_e = nc.declare_dram_parameter("out", [18, H, W], F32, isOutput=True)

    wscr = nc.dram_tensor("wscratch", [H, 4, W], BF16)
    iscr = nc.dram_tensor("iscratch", [H, 2, W], I16)
    if KDBG:
        dbg_warp = nc.declare_dram_parameter("dbg_warp", [C, H, W], F32, isOutput=True)
        dbg_idx = nc.declare_dram_parameter("dbg_idx", [128, 4, NPIX // 16], I16, isOutput=True)
        dbg_g = nc.declare_dram_parameter("dbg_g", [C, 4, CPIX], F32, isOutput=True)
        dbg_w = nc.declare_dram_parameter("dbg_w", [C, 4, CPIX], F32, isOutput=True)

    with TileContext(nc) as tc:
        with (
            tc.tile_pool(name="per", bufs=1) as per,
            tc.tile_pool(name="rbp", bufs=1) as rbp,
            tc.tile_pool(name="gp", bufs=2) as gp,
            tc.tile_pool(name="wp", bufs=1) as wp,
            tc.tile_pool(name="wfp", bufs=1) as wfp,
            tc.tile_pool(name="lp", bufs=2) as lp,
            tc.tile_pool(name="ltp", bufs=1) as ltp,
            tc.tile_pool(name="tkp", bufs=3) as tkp,
            tc.tile_pool(name="osp", bufs=2) as osp,
            tc.tile_pool(name="cps", bufs=4, space="PSUM") as cps,
            tc.tile_pool(name="wps", bufs=2, space="PSUM") as wps,
        ):
            # ---------- persistent loads ----------
            L = per.tile([C, NPIX], BF16, tag="L", name="L")
            nc.gpsimd.dma_start(L[:], left_e[:].rearrange("c h w -> c (h w)"))

            ones_tile = per.tile([65, 128], BF16, tag="ones", name="ones")
            nc.gpsimd.dma_start(ones_tile[0:1, :], ones_e[:])
            nc.gpsimd.dma_start(ones_tile[64:65, :], ones_e[:])
            ones_sb = ones_tile[0:1, :]
            ones64 = ones_tile[64:65, :]
            selw_sb = per.tile([128, 9, 18], BF16, tag="selw", name="selw")
            nc.gpsimd.dma_start(
                selw_sb[:], selw_e[:].rearrange("k c j -> c k j"))

            # ---------- tokens: [w-part, rank h, c] + zero stripe ----------
            # rank r holds [row r-1 | row r] (512B); rows duplicated across slots
            tok = per.tile([128, H + 3, 2, C], BF16, tag="tok", name="tok")
            nc.gpsimd.memset(tok[:, 0, 0, :], 0)
            nc.gpsimd.memset(tok[:, H, 1, :], 0)
            nc.gpsimd.memset(tok[:, H + 1, :, :], 0)
            nc.gpsimd.memset(tok[:, H + 2, :, :], 0)
            for rh in ([] if KSTAGE == 'tok0' else [0, 1, 2, 3]):
                rbh = rbp.tile([C, NPIX // 4], BF16, tag="rbh", name="rbh")
                nc.gpsimd.dma_start(
                    rbh[:],
                    right_e[:, rh * (H // 4):(rh + 1) * (H // 4), :]
                    .rearrange("c h w -> c (h w)"))
                nc.sync.dma_start_transpose(
                    tok[:, 1 + rh * (H // 4):1 + (rh + 1) * (H // 4), 0, :], rbh[:])
                nc.sync.dma_start_transpose(
                    tok[:, rh * (H // 4):(rh + 1) * (H // 4), 1, :], rbh[:])

            # ---------- params on [96h, 128w], all in one sliced tile -------
            pt = per.tile([H, 25, W], F32, tag="pt", name="pt")
            slot = {}

            def P(tag):
                if tag not in slot:
                    slot[tag] = len(slot)
                    assert len(slot) <= 25, slot
                return pt[:, slot[tag], :]

            def TT(dst, a, b, op):
                nc.vector.tensor_tensor(out=dst, in0=a, in1=b, op=op)

            def TS(dst, a, s1, s2, op0, op1=None):
                kw = {} if op1 is None else {"op1": op1}
                nc.vector.tensor_scalar(out=dst, in0=a, scalar1=s1, scalar2=s2,
                                        op0=op0, **kw)

            nc.sync.dma_start(P("fxin"), flow_e[0])
            nc.sync.dma_start(P("fyin"), flow_e[1])
            nc.sync.dma_start(P("gx"), grid_e[0])
            nc.sync.dma_start(P("gy"), grid_e[1])

            TT(P("x"), P("gx"), P("fxin"), ALU.add)
            TT(P("y"), P("gy"), P("fyin"), ALU.add)
            MAGIC = 12582912.0  # 1.5*2**23 (ulp=1): floor via round(x-0.5)
            TS(P("x0"), P("x"), -0.5, MAGIC, ALU.add, ALU.add)
            TS(P("x0"), P("x0"), -MAGIC, None, ALU.add)
            TS(P("y0"), P("y"), -0.5, MAGIC, ALU.add, ALU.add)
            TS(P("y0"), P("y0"), -MAGIC, None, ALU.add)
            TT(P("fx"), P("x"), P("x0"), ALU.subtract)
            TT(P("fy"), P("y"), P("y0"), ALU.subtract)

            def valid(src, lo, hi, tag):
                TS(P(tag + "a"), src, float(lo), None, ALU.is_ge)
                TS(P("vtmp"), src, float(hi), None, ALU.is_le)
                TT(P(tag + "a"), P(tag + "a"), P("vtmp"), ALU.mult)
                return P(tag + "a")

            vx0 = valid(P("x0"), 0, 127, "vx0")
            vx1 = valid(P("x0"), -1, 126, "vx1")
            vy0 = valid(P("y0"), 0, 95, "vy0")
            vy1 = valid(P("y0"), -1, 94, "vy1")

            s = 1.0 / 64.0
            TS(P("wx0"), P("fx"), -s, s, ALU.mult, ALU.add)
            TS(P("wx1"), P("fx"), s, None, ALU.mult)
            TS(P("wy0"), P("fy"), -1.0, 1.0, ALU.mult, ALU.add)
            TS(P("wy1"), P("fy"), 1.0, None, ALU.mult)

            wmap = per.tile([H, 4, W], F32, tag="wmap", name="wmap")
            TT(wmap[:, 0, :], P("wx0"), P("wy0"), ALU.mult)
            TT(wmap[:, 1, :], P("wx0"), P("wy1"), ALU.mult)
            TT(wmap[:, 2, :], P("wx1"), P("wy0"), ALU.mult)
            TT(wmap[:, 3, :], P("wx1"), P("wy1"), ALU.mult)

            TS(P("xc0"), P("x0"), 0.0, 127.0, ALU.max, ALU.min)
            TS(P("xc1"), P("x0"), 1.0, 0.0, ALU.add, ALU.max)
            TS(P("xc1"), P("xc1"), 127.0, None, ALU.min)
            # pair row base: (y0+1)*128, pair-validity y0 in [-1, 95]
            TS(P("yb0"), P("y0"), 1.0, 128.0, ALU.add, ALU.mult)
            TS(P("pya"), P("y0"), -1.0, None, ALU.is_ge)
            TS(P("vtmp"), P("y0"), 95.0, None, ALU.is_le)
            TT(P("pya"), P("pya"), P("vtmp"), ALU.mult)

            ZP = 97 * 128  # zero-pair token (ranks 97,98 are zero)
            imap = per.tile([H, 2, W], F32, tag="imap", name="imap")
            for n, (xcn, vxn) in enumerate([("xc0", vx0), ("xc1", vx1)]):
                TT(P("bn"), P("yb0"), P(xcn), ALU.add)
                TT(P("vn"), P("pya"), vxn, ALU.mult)
                TS(P("bn"), P("bn"), -float(ZP), None, ALU.add)
                TT(P("bn"), P("bn"), P("vn"), ALU.mult)
                TS(imap[:, n, :], P("bn"), float(ZP), None, ALU.add)

            # cast + micro-permute w -> (q*8 + wh) in one copy
            imap16t = per.tile([H, 2, W], I16, tag="imap16t", name="imap16t")
            nc.vector.tensor_copy(
                imap16t[:].rearrange("h n (q wh) -> h n q wh", q=16),
                imap[:].rearrange("h n (wh q) -> h n q wh", wh=8))

            # weights/indices via DRAM rearrange bounce
            nc.gpsimd.dma_start(wscr[:], wmap[:])
            nc.sync.dma_start(iscr[:], imap16t[:])

            idx_all = per.tile([128, 2, NPIX // 16], I16, tag="idx", name="idx")
            # row p=(ph*16+q) holds, at (n, j), idx of pixel j*16 + q
            # j = h*8 + wh ; pixel = h*128 + wh*16 + q
            # iscr[h, n, q*8 + wh] = idx_n(h, wh*16 + q)
            for n in range(2):
                nc.sync.dma_start(
                    idx_all[0:16, n].rearrange("q (h wh) -> q h wh", wh=8),
                    iscr[:, n].rearrange("h (q wh) -> q h wh", q=16))
            for ph in range(1, 8):
                nc.sync.dma_start(idx_all[ph * 16:(ph + 1) * 16], idx_all[0:16])

            # ---------- per-chunk pipeline ----------
            for ci in range(NCH):
                p0 = ci * CPIX
                h0 = ci * CROWS
                # gathers
                G4 = gp.tile([128, 4, CPIX], BF16, tag="G", name="G%d" % ci)
                for n in range(2 if KSTAGE != 'noga' else 0):
                    nc.gpsimd.dma_gather(
                        out_ap=G4[:, 2 * n:2 * n + 2, :],
                        in_ap=tok[:].rearrange("p h s c -> p (h s c)"),
                        idxs_ap=idx_all[:, n, ci * (CPIX // 16):(ci + 1) * (CPIX // 16)],
                        num_idxs=CPIX, num_idxs_reg=CPIX, elem_size=2 * C,
                        transpose=True, sbuf_tokens_per_rank=128,
                        sbuf_free_dim_per_rank=C * 4, single_packet=False)

                if KSTAGE in ('tok', 'idx', 'noga'):
                    osbg2 = osp.tile([18, CPIX], BF16, tag="osb", name="osbg2%d" % ci)
                    if KSTAGE in ('tok', 'noga'):
                        nc.vector.tensor_copy(
                            osbg2[:], tok[0:18, h0:h0 + CROWS, :]
                            .rearrange("p h c -> p (h c)"))
                    else:
                        nc.vector.tensor_copy(
                            osbg2[:, 0:768],
                            idx_all[0:18, 0, :].bitcast(BF16))
                    nc.gpsimd.dma_start(
                        out_e[:, h0:h0 + CROWS, :].rearrange("j h w -> j (h w)"),
                        osbg2[:])
                    continue
                if KSTAGE == 'gather':
                    osbg = osp.tile([18, CPIX], BF16, tag="osb", name="osbg%d" % ci)
                    nc.vector.tensor_copy(osbg[:], G4[0:18, 0, :])
                    nc.gpsimd.dma_start(
                        out_e[:, h0:h0 + CROWS, :].rearrange("j h w -> j (h w)"),
                        osbg[:])
                    continue
                # weight maps for this chunk: [1, 4*CPIX] bf16 <- DRAM
                wfc = wfp.tile([65, 2, CPIX], BF16, tag="wfc", name="wfc%d" % ci)
                for n in range(4):
                    nc.sync.dma_start(
                        wfc[(n % 2) * 64:(n % 2) * 64 + 1, n // 2]
                        .rearrange("o (h w) -> o h w", w=W),
                        wscr[h0:h0 + CROWS, n])
                W4 = wp.tile([128, 4, CPIX], BF16, tag="W", name="W%d" % ci)
                for n in range(4):
                    for hf in range(2):
                        ps = wps.tile([128, CPIX // 2], F32, tag="wps",
                                      name="wps")
                        for sb in range(2):
                            o0 = hf * (CPIX // 2) + sb * SUB
                            nc.tensor.matmul(
                                ps[:, sb * SUB:(sb + 1) * SUB],
                                ones_sb if n % 2 == 0 else ones64,
                                wfc[(n % 2) * 64:(n % 2) * 64 + 1, n // 2,
                                    o0:o0 + SUB])
                        nc.scalar.copy(
                            W4[:, n, hf * (CPIX // 2):(hf + 1) * (CPIX // 2)],
                            ps[:])

                # lerp -> padded tiles
                pe = lp.tile([128, CROWS, 136], BF16, tag="pe", name="pe%d" % ci)
                po = lp.tile([128, CROWS, 138], BF16, tag="po", name="po%d" % ci)
                t0 = ltp.tile([128, CPIX], BF16, tag="lt0", name="lt0")
                t1 = ltp.tile([128, CPIX], BF16, tag="lt1", name="lt1")
                TT(t0[:], G4[:, 0, :], W4[:, 0, :], ALU.mult)
                TT(t1[:], G4[:, 1, :], W4[:, 1, :], ALU.mult)
                TT(t0[:], t0[:], t1[:], ALU.add)
                TT(t1[:], G4[:, 2, :], W4[:, 2, :], ALU.mult)
                TT(t0[:], t0[:], t1[:], ALU.add)
                TT(t1[:], G4[:, 3, :], W4[:, 3, :], ALU.mult)
                nc.vector.tensor_tensor(
                    out=pe[:, :, 4:132],
                    in0=t0[:].rearrange("c (h w) -> c h w", w=W),
                    in1=t1[:].rearrange("c (h w) -> c h w", w=W), op=ALU.add)
                for j in range(4):
                    nc.vector.tensor_copy(pe[:, :, j:j + 1], pe[:, :, 4:5])
                    nc.vector.tensor_copy(pe[:, :, 132 + j:133 + j],
                                          pe[:, :, 131:132])
                nc.sync.dma_start(po[:, :, 1:137], pe[:, :, 0:136])
                if KDBG:
                    nc.gpsimd.dma_start(
                        dbg_warp[:, h0:h0 + CROWS, :], pe[:, :, 4:132])
                    if ci == 0:
                        nc.gpsimd.dma_start(dbg_idx[:], idx_all[:])
                        nc.gpsimd.dma_start(dbg_g[:], G4[:])
                        nc.gpsimd.dma_start(dbg_w[:], W4[:])

                # taps + selector matmuls
                osb = osp.tile([18, CPIX], F32, tag="osb", name="osb%d" % ci)
                if KSTAGE == 'warp':
                    nc.vector.tensor_copy(
                        osb[:].rearrange("c (h w) -> c h w", w=W),
                        pe[0:18, :, 4:132])
                    nc.gpsimd.dma_start(
                        out_e[:, h0:h0 + CROWS, :].rearrange("j h w -> j (h w)"),
                        osb[:])
                    continue
                for tc2 in range(2):
                    q0 = tc2 * (CPIX // 2)
                    lsl = L[:, p0 + q0: p0 + q0 + CPIX // 2]
                    pss = [cps.tile([128, SUB], F32, tag="cpsum",
                                    name="cps%d_%d_%d" % (ci, tc2, i))
                           for i in range(2)]
                    korder = [0, 2, 4, 6, 8, 1, 3, 5, 7]
                    for ki, k in enumerate(korder):
                        tk = tkp.tile([128, CPIX // 2], BF16, tag="tk",
                                      name="tk")
                        if k % 2 == 0:
                            rw = pe[:, tc2 * 8:(tc2 + 1) * 8, k:k + 128]
                        else:
                            rw = po[:, tc2 * 8:(tc2 + 1) * 8, k + 1:k + 129]
                        nc.vector.tensor_tensor(
                            out=tk[:].rearrange("c (h w) -> c h w", w=W),
                            in0=lsl.rearrange("c (h w) -> c h w", w=W),
                            in1=rw, op=ALU.mult)
                        for sb in range(2):
                            nc.tensor.matmul(
                                pss[sb][0:18, :], selw_sb[:, k, :],
                                tk[:, sb * SUB:(sb + 1) * SUB],
                                start=(ki == 0), stop=(ki == 8))
                    for sb in range(2):
                        nc.scalar.copy(
                            osb[:, q0 + sb * SUB:q0 + (sb + 1) * SUB],
                            pss[sb][0:18, :])
                nc.sync.dma_start(
                    out_e[:, h0:h0 + CROWS, :].rearrange("j h w -> j (h w)"),
                    osb[:])
    nc.compile()
    return nc


_NC_CACHE = {}


def _get_nc():
    if "nc" not in _NC_CACHE:
        _NC_CACHE["nc"] = build_nc()
    return _NC_CACHE["nc"]


def _consts():
    yy, xx = np.meshgrid(np.arange(H, dtype=np.float32),
                         np.arange(W, dtype=np.float32), indexing="ij")
    grid = np.stack([xx, yy], axis=0).astype(np.float32)  # [2, H, W] (x, y)
    ones1 = np.ones((1, 128), np.float32)
    selw = np.zeros((9, 128, 18), np.float32)
    for k in range(9):
        for c in range(128):
            g = c // 64
            selw[k, c, g * 9 + k] = 1.0
    return grid, ones1, selw


def _in_maps(left_features, right_features, flow):
    grid, ones1, selw = _consts()
    in_maps = []
    for core in range(8):
        b, half = core // 2, core % 2
        in_maps.append({
            "left": np.ascontiguousarray(
                left_features[b, half * 128:(half + 1) * 128]).astype(np.float32),
            "right": np.ascontiguousarray(
                right_features[b, half * 128:(half + 1) * 128]).astype(np.float32),
            "flow": np.ascontiguousarray(flow[b]).astype(np.float32),
            "grid": grid, "ones1": ones1, "selw": selw,
        })
    return in_maps


def kernel(left_features, right_features, flow):
    nc = _get_nc()
    res = run_bass_kernel_spmd(nc, _in_maps(left_features, right_features, flow),
                               core_ids=list(range(8)))
    out = np.zeros((4, 36, H, W), np.float32)
    for core in range(8):
        b, half = core // 2, core % 2
        out[b, half * 18:(half + 1) * 18] = res.results[core]["out"]
    return out


# revision 43
# speedup vs baseline: 1.0033x; 1.0026x over previous
"""Trainium2 Bass kernel: AdaptiveGroupCorrelationLayer.

Sharding: 8 cores = 4 batches x 2 channel-halves (128 ch each = 2 groups of 64).
Per core:
  1. Load left/right [128c, 96, 128] f32 -> bf16 (cast-DMA), flow/grid [96,128].
  2. Param math on [96h, 128w]: bilinear coords, 4 weights (1/64 folded),
     4 gather indices (invalid neighbors -> zero-token 12288).
  3. right -> token layout [w, h, c] via dma_start_transpose; zero pad stripe.
  4. 4x dma_gather (SBUF-source, transpose mode) -> G_n [128c, pix] bf16.
  5. Weights broadcast [1,pix] -> [128,pix] via PE outer product + ACT copy.
  6. Lerp: R_w = sum w_n*G_n (7 DVE TT ops), written edge-padded (even+odd).
  7. Taps: T_k = L * shift_k(R_w); masked-selector matmuls accumulate
     corr[(g,k), pix] in PSUM [18, 512] chunks over the 9 taps.
  8. ACT copy -> bf16, cast-DMA out to [18, 96, 128] f32.
"""
import sys, os
sys.path.insert(0, '/opt/trn_rl_repo')
import numpy as np
KDBG = bool(os.environ.get('KDBG'))
KSTAGE = os.environ.get('KSTAGE', 'full')

import concourse.bass as bass
from concourse import bacc
import concourse.mybir as mybir
from concourse.tile import TileContext
from concourse.bass_utils import run_bass_kernel_spmd

F32 = mybir.dt.float32
BF16 = mybir.dt.bfloat16
I16 = mybir.dt.int16
ALU = mybir.AluOpType

H, W, C = 96, 128, 128       # per-core shard
NPIX = H * W                 # 12288
NCH = 6                      # h-chunks
CROWS = H // NCH             # 16 rows per chunk
CPIX = CROWS * W             # 2048 px per chunk
ZTOK = NPIX                  # zero token index
SUB = 512


def build_nc():
    nc = bacc.Bacc()
    left_e = nc.declare_dram_parameter("left", [C, H, W], F32, isOutput=False)
    right_e = nc.declare_dram_parameter("right", [C, H, W], F32, isOutput=False)
    flow_e = nc.declare_dram_parameter("flow", [2, H, W], F32, isOutput=False)
    grid_e = nc.declare_dram_parameter("grid", [2, H, W], F32, isOutput=False)
    ones_e = nc.declare_dram_parameter("ones1", [1, 128], F32, isOutput=False)
    selw_e = nc.declare_dram_parameter("selw", [9, 128, 18], F32, isOutput=False)
    out_e = nc.declare_dram_parameter("out", [18, H, W], F32, isOutput=True)

    wscr = nc.dram_tensor("wscratch", [H, 4, W], BF16)
    iscr = nc.dram_tensor("iscratch", [H, 2, W], I16)
    if KDBG:
        dbg_warp = nc.declare_dram_parameter("dbg_warp", [C, H, W], F32, isOutput=True)
        dbg_idx = nc.declare_dram_parameter("dbg_idx", [128, 4, NPIX // 16], I16, isOutput=True)
        dbg_g = nc.declare_dram_parameter("dbg_g", [C, 4, CPIX], F32, isOutput=True)
        dbg_w = nc.declare_dram_parameter("dbg_w", [C, 4, CPIX], F32, isOutput=True)

    with TileContext(nc) as tc:
        with (
            tc.tile_pool(name="per", bufs=1) as per,
            tc.tile_pool(name="rbp", bufs=1) as rbp,
            tc.tile_pool(name="gp", bufs=2) as gp,
            tc.tile_pool(name="wp", bufs=1) as wp,
            tc.tile_pool(name="wfp", bufs=1) as wfp,
            tc.tile_pool(name="lp", bufs=2) as lp,
            tc.tile_pool(name="ltp", bufs=1) as ltp,
            tc.tile_pool(name="tkp", bufs=4) as tkp,
            tc.tile_pool(name="osp", bufs=2) as osp,
            tc.tile_pool(name="cps", bufs=4, space="PSUM") as cps,
            tc.tile_pool(name="wps", bufs=2, space="PSUM") as wps,
        ):
            # ---------- persistent loads ----------
            L = per.tile([C, NPIX], BF16, tag="L", name="L")
            nc.gpsimd.dma_start(L[:], left_e[:].rearrange("c h w -> c (h w)"))

            ones_tile = per.tile([65, 128], BF16, tag="ones", name="ones")
            nc.gpsimd.dma_start(ones_tile[0:1, :], ones_e[:])
            nc.gpsimd.dma_start(ones_tile[64:65, :], ones_e[:])
            ones_sb = ones_tile[0:1, :]
            ones64 = ones_tile[64:65, :]
            selw_sb = per.tile([128, 9, 18], BF16, tag="selw", name="selw")
            nc.gpsimd.dma_start(
                selw_sb[:], selw_e[:].rearrange("k c j -> c k j"))

            # ---------- tokens: [w-part, rank h, c] + zero stripe ----------
            # rank r holds [row r-1 | row r] (512B); rows duplicated across slots
            tok = per.tile([128, H + 3, 2, C], BF16, tag="tok", name="tok")
            nc.gpsimd.memset(tok[:, 0, 0, :], 0)
            nc.gpsimd.memset(tok[:, H, 1, :], 0)
            nc.gpsimd.memset(tok[:, H + 1, :, :], 0)
            nc.gpsimd.memset(tok[:, H + 2, :, :], 0)
            for rh in ([] if KSTAGE == 'tok0' else [0, 1, 2, 3]):
                rbh = rbp.tile([C, NPIX // 4], BF16, tag="rbh", name="rbh")
                nc.gpsimd.dma_start(
                    rbh[:],
                    right_e[:, rh * (H // 4):(rh + 1) * (H // 4), :]
                    .rearrange("c h w -> c (h w)"))
                nc.sync.dma_start_transpose(
                    tok[:, 1 + rh * (H // 4):1 + (rh + 1) * (H // 4), 0, :], rbh[:])
                nc.sync.dma_start_transpose(
                    tok[:, rh * (H // 4):(rh + 1) * (H // 4), 1, :], rbh[:])

            # ---------- params on [96h, 128w], all in one sliced tile -------
            pt = per.tile([H, 25, W], F32, tag="pt", name="pt")
            slot = {}

            def P(tag):
                if tag not in slot:
                    slot[tag] = len(slot)
                    assert len(slot) <= 25, slot
                return pt[:, slot[tag], :]

            def TT(dst, a, b, op):
                nc.vector.tensor_tensor(out=dst, in0=a, in1=b, op=op)

            def TS(dst, a, s1, s2, op0, op1=None):
                kw = {} if op1 is None else {"op1": op1}
                nc.vector.tensor_scalar(out=dst, in0=a, scalar1=s1, scalar2=s2,
                                        op0=op0, **kw)

            nc.sync.dma_start(P("fxin"), flow_e[0])
            nc.sync.dma_start(P("fyin"), flow_e[1])
            nc.sync.dma_start(P("gx"), grid_e[0])
            nc.sync.dma_start(P("gy"), grid_e[1])

            TT(P("x"), P("gx"), P("fxin"), ALU.add)
            TT(P("y"), P("gy"), P("fyin"), ALU.add)
            MAGIC = 12582912.0  # 1.5*2**23 (ulp=1): floor via round(x-0.5)
            TS(P("x0"), P("x"), -0.5, MAGIC, ALU.add, ALU.add)
            TS(P("x0"), P("x0"), -MAGIC, None, ALU.add)
            TS(P("y0"), P("y"), -0.5, MAGIC, ALU.add, ALU.add)
            TS(P("y0"), P("y0"), -MAGIC, None, ALU.add)
            TT(P("fx"), P("x"), P("x0"), ALU.subtract)
            TT(P("fy"), P("y"), P("y0"), ALU.subtract)

            def valid(src, lo, hi, tag):
                TS(P(tag + "a"), src, float(lo), None, ALU.is_ge)
                TS(P("vtmp"), src, float(hi), None, ALU.is_le)
                TT(P(tag + "a"), P(tag + "a"), P("vtmp"), ALU.mult)
                return P(tag + "a")

            vx0 = valid(P("x0"), 0, 127, "vx0")
            vx1 = valid(P("x0"), -1, 126, "vx1")
            vy0 = valid(P("y0"), 0, 95, "vy0")
            vy1 = valid(P("y0"), -1, 94, "vy1")

            s = 1.0 / 64.0
            TS(P("wx0"), P("fx"), -s, s, ALU.mult, ALU.add)
            TS(P("wx1"), P("fx"), s, None, ALU.mult)
            TS(P("wy0"), P("fy"), -1.0, 1.0, ALU.mult, ALU.add)
            TS(P("wy1"), P("fy"), 1.0, None, ALU.mult)

            wmap = per.tile([H, 4, W], F32, tag="wmap", name="wmap")
            TT(wmap[:, 0, :], P("wx0"), P("wy0"), ALU.mult)
            TT(wmap[:, 1, :], P("wx0"), P("wy1"), ALU.mult)
            TT(wmap[:, 2, :], P("wx1"), P("wy0"), ALU.mult)
            TT(wmap[:, 3, :], P("wx1"), P("wy1"), ALU.mult)

            TS(P("xc0"), P("x0"), 0.0, 127.0, ALU.max, ALU.min)
            TS(P("xc1"), P("x0"), 1.0, 0.0, ALU.add, ALU.max)
            TS(P("xc1"), P("xc1"), 127.0, None, ALU.min)
            # pair row base: (y0+1)*128, pair-validity y0 in [-1, 95]
            TS(P("yb0"), P("y0"), 1.0, 128.0, ALU.add, ALU.mult)
            TS(P("pya"), P("y0"), -1.0, None, ALU.is_ge)
            TS(P("vtmp"), P("y0"), 95.0, None, ALU.is_le)
            TT(P("pya"), P("pya"), P("vtmp"), ALU.mult)

            ZP = 97 * 128  # zero-pair token (ranks 97,98 are zero)
            imap = per.tile([H, 2, W], F32, tag="imap", name="imap")
            for n, (xcn, vxn) in enumerate([("xc0", vx0), ("xc1", vx1)]):
                TT(P("bn"), P("yb0"), P(xcn), ALU.add)
                TT(P("vn"), P("pya"), vxn, ALU.mult)
                TS(P("bn"), P("bn"), -float(ZP), None, ALU.add)
                TT(P("bn"), P("bn"), P("vn"), ALU.mult)
                TS(imap[:, n, :], P("bn"), float(ZP), None, ALU.add)

            # cast + micro-permute w -> (q*8 + wh) in one copy
            imap16t = per.tile([H, 2, W], I16, tag="imap16t", name="imap16t")
            nc.vector.tensor_copy(
                imap16t[:].rearrange("h n (q wh) -> h n q wh", q=16),
                imap[:].rearrange("h n (wh q) -> h n q wh", wh=8))

            # weights/indices via DRAM rearrange bounce
            nc.gpsimd.dma_start(wscr[:], wmap[:])
            nc.sync.dma_start(iscr[:], imap16t[:])

            idx_all = per.tile([128, 2, NPIX // 16], I16, tag="idx", name="idx")
            # row p=(ph*16+q) holds, at (n, j), idx of pixel j*16 + q
            # j = h*8 + wh ; pixel = h*128 + wh*16 + q
            # iscr[h, n, q*8 + wh] = idx_n(h, wh*16 + q)
            for n in range(2):
                nc.sync.dma_start(
                    idx_all[0:16, n].rearrange("q (h wh) -> q h wh", wh=8),
                    iscr[:, n].rearrange("h (q wh) -> q h wh", q=16))
            for ph in range(1, 8):
                nc.sync.dma_start(idx_all[ph * 16:(ph + 1) * 16], idx_all[0:16])

            # ---------- per-chunk pipeline ----------
            for ci in range(NCH):
                p0 = ci * CPIX
                h0 = ci * CROWS
                # gathers
                G4 = gp.tile([128, 4, CPIX], BF16, tag="G", name="G%d" % ci)
                for n in range(2 if KSTAGE != 'noga' else 0):
                    nc.gpsimd.dma_gather(
                        out_ap=G4[:, 2 * n:2 * n + 2, :],
                        in_ap=tok[:].rearrange("p h s c -> p (h s c)"),
                        idxs_ap=idx_all[:, n, ci * (CPIX // 16):(ci + 1) * (CPIX // 16)],
                        num_idxs=CPIX, num_idxs_reg=CPIX, elem_size=2 * C,
                        transpose=True, sbuf_tokens_per_rank=128,
                        sbuf_free_dim_per_rank=C * 4, single_packet=False)

                if KSTAGE in ('tok', 'idx', 'noga'):
                    osbg2 = osp.tile([18, CPIX], BF16, tag="osb", name="osbg2%d" % ci)
                    if KSTAGE in ('tok', 'noga'):
                        nc.vector.tensor_copy(
                            osbg2[:], tok[0:18, h0:h0 + CROWS, :]
                            .rearrange("p h c -> p (h c)"))
                    else:
                        nc.vector.tensor_copy(
                            osbg2[:, 0:768],
                            idx_all[0:18, 0, :].bitcast(BF16))
                    nc.gpsimd.dma_start(
                        out_e[:, h0:h0 + CROWS, :].rearrange("j h w -> j (h w)"),
                        osbg2[:])
                    continue
                if KSTAGE == 'gather':
                    osbg = osp.tile([18, CPIX], BF16, tag="osb", name="osbg%d" % ci)
                    nc.vector.tensor_copy(osbg[:], G4[0:18, 0, :])
                    nc.gpsimd.dma_start(
                        out_e[:, h0:h0 + CROWS, :].rearrange("j h w -> j (h w)"),
                        osbg[:])
                    continue
                # weight maps for this chunk: [1, 4*CPIX] bf16 <- DRAM
                wfc = wfp.tile([65, 2, CPIX], BF16, tag="wfc", name="wfc%d" % ci)
                for n in range(4):
                    nc.sync.dma_start(
                        wfc[(n % 2) * 64:(n % 2) * 64 + 1, n // 2]
                        .rearrange("o (h w) -> o h w", w=W),
                        wscr[h0:h0 + CROWS, n])
                W4 = wp.tile([128, 4, CPIX], BF16, tag="W", name="W%d" % ci)
                for n in range(4):
                    for hf in range(2):
                        ps = wps.tile([128, CPIX // 2], F32, tag="wps",
                                      name="wps")
                        for sb in range(2):
                            o0 = hf * (CPIX // 2) + sb * SUB
                            nc.tensor.matmul(
                                ps[:, sb * SUB:(sb + 1) * SUB],
                                ones_sb if n % 2 == 0 else ones64,
                                wfc[(n % 2) * 64:(n % 2) * 64 + 1, n // 2,
                                    o0:o0 + SUB])
                        nc.scalar.copy(
                            W4[:, n, hf * (CPIX // 2):(hf + 1) * (CPIX // 2)],
                            ps[:])

                # lerp -> padded tiles
                pe = lp.tile([128, CROWS, 136], BF16, tag="pe", name="pe%d" % ci)
                po = lp.tile([128, CROWS, 138], BF16, tag="po", name="po%d" % ci)
                t0 = ltp.tile([128, CPIX], BF16, tag="lt0", name="lt0")
                t1 = ltp.tile([128, CPIX], BF16, tag="lt1", name="lt1")
                TT(t0[:], G4[:, 0, :], W4[:, 0, :], ALU.mult)
                TT(t1[:], G4[:, 1, :], W4[:, 1, :], ALU.mult)
                TT(t0[:], t0[:], t1[:], ALU.add)
                TT(t1[:], G4[:, 2, :], W4[:, 2, :], ALU.mult)
                TT(t0[:], t0[:], t1[:], ALU.add)
                TT(t1[:], G4[:, 3, :], W4[:, 3, :], ALU.mult)
                nc.vector.tensor_tensor(
                    out=pe[:, :, 4:132],
                    in0=t0[:].rearrange("c (h w) -> c h w", w=W),
                    in1=t1[:].rearrange("c (h w) -> c h w", w=W), op=ALU.add)
                for j in range(4):
                    nc.vector.tensor_copy(pe[:, :, j:j + 1], pe[:, :, 4:5])
                    nc.vector.tensor_copy(pe[:, :, 132 + j:133 + j],
                                          pe[:, :, 131:132])
                nc.sync.dma_start(po[:, :, 1:137], pe[:, :, 0:136])
                if KDBG:
                    nc.gpsimd.dma_start(
                        dbg_warp[:, h0:h0 + CROWS, :], pe[:, :, 4:132])
                    if ci == 0:
                        nc.gpsimd.dma_start(dbg_idx[:], idx_all[:])
                        nc.gpsimd.dma_start(dbg_g[:], G4[:])
                        nc.gpsimd.dma_start(dbg_w[:], W4[:])

                # taps + selector matmuls
                osb = osp.tile([18, CPIX], F32, tag="osb", name="osb%d" % ci)
                if KSTAGE == 'warp':
                    nc.vector.tensor_copy(
                        osb[:].rearrange("c (h w) -> c h w", w=W),
                        pe[0:18, :, 4:132])
                    nc.gpsimd.dma_start(
                        out_e[:, h0:h0 + CROWS, :].rearrange("j h w -> j (h w)"),
                        osb[:])
                    continue
                for tc2 in range(2):
                    q0 = tc2 * (CPIX // 2)
                    lsl = L[:, p0 + q0: p0 + q0 + CPIX // 2]
                    pss = [cps.tile([128, SUB], F32, tag="cpsum",
                                    name="cps%d_%d_%d" % (ci, tc2, i))
                           for i in range(2)]
                    korder = [0, 2, 4, 6, 8, 1, 3, 5, 7]
                    for ki, k in enumerate(korder):
                        tk = tkp.tile([128, CPIX // 2], BF16, tag="tk",
                                      name="tk")
                        if k % 2 == 0:
                            rw = pe[:, tc2 * 8:(tc2 + 1) * 8, k:k + 128]
                        else:
                            rw = po[:, tc2 * 8:(tc2 + 1) * 8, k + 1:k + 129]
                        nc.vector.tensor_tensor(
                            out=tk[:].rearrange("c (h w) -> c h w", w=W),
                            in0=lsl.rearrange("c (h w) -> c h w", w=W),
                            in1=rw, op=ALU.mult)
                        for sb in range(2):
                            nc.tensor.matmul(
                                pss[sb][0:18, :], selw_sb[:, k, :],
                                tk[:, sb * SUB:(sb + 1) * SUB],
                                start=(ki == 0), stop=(ki == 8))
                    for sb in range(2):
                        nc.scalar.copy(
                            osb[:, q0 + sb * SUB:q0 + (sb + 1) * SUB],
                            pss[sb][0:18, :])
                nc.sync.dma_start(
                    out_e[:, h0:h0 + CROWS, :].rearrange("j h w -> j (h w)"),
                    osb[:])
    nc.compile()
    return nc


_NC_CACHE = {}


def _get_nc():
    if "nc" not in _NC_CACHE:
        _NC_CACHE["nc"] = build_nc()
    return _NC_CACHE["nc"]


def _consts():
    yy, xx = np.meshgrid(np.arange(H, dtype=np.float32),
                         np.arange(W, dtype=np.float32), indexing="ij")
    grid = np.stack([xx, yy], axis=0).astype(np.float32)  # [2, H, W] (x, y)
    ones1 = np.ones((1, 128), np.float32)
    selw = np.zeros((9, 128, 18), np.float32)
    for k in range(9):
        for c in range(128):
            g = c // 64
            selw[k, c, g * 9 + k] = 1.0
    return grid, ones1, selw


def _in_maps(left_features, right_features, flow):
    grid, ones1, selw = _consts()
    in_maps = []
    for core in range(8):
        b, half = core // 2, core % 2
        in_maps.append({
            "left": np.ascontiguousarray(
                left_features[b, half * 128:(half + 1) * 128]).astype(np.float32),
            "right": np.ascontiguousarray(
                right_features[b, half * 128:(half + 1) * 128]).astype(np.float32),
            "flow": np.ascontiguousarray(flow[b]).astype(np.float32),
            "grid": grid, "ones1": ones1, "selw": selw,
        })
    return in_maps


def kernel(left_features, right_features, flow):
    nc = _get_nc()
    res = run_bass_kernel_spmd(nc, _in_maps(left_features, right_features, flow),
                               core_ids=list(range(8)))
    out = np.zeros((4, 36, H, W), np.float32)
    for core in range(8):
        b, half = core // 2, core % 2
        out[b, half * 18:(half + 1) * 18] = res.results[core]["out"]
    return out


# revision 44
# speedup vs baseline: 1.0418x; 1.0384x over previous
"""Trainium2 Bass kernel: AdaptiveGroupCorrelationLayer.

Sharding: 8 cores = 4 batches x 2 channel-halves (128 ch each = 2 groups of 64).
Per core:
  1. Load left/right [128c, 96, 128] f32 -> bf16 (cast-DMA), flow/grid [96,128].
  2. Param math on [96h, 128w]: bilinear coords, 4 weights (1/64 folded),
     4 gather indices (invalid neighbors -> zero-token 12288).
  3. right -> token layout [w, h, c] via dma_start_transpose; zero pad stripe.
  4. 4x dma_gather (SBUF-source, transpose mode) -> G_n [128c, pix] bf16.
  5. Weights broadcast [1,pix] -> [128,pix] via PE outer product + ACT copy.
  6. Lerp: R_w = sum w_n*G_n (7 DVE TT ops), written edge-padded (even+odd).
  7. Taps: T_k = L * shift_k(R_w); masked-selector matmuls accumulate
     corr[(g,k), pix] in PSUM [18, 512] chunks over the 9 taps.
  8. ACT copy -> bf16, cast-DMA out to [18, 96, 128] f32.
"""
import sys, os
sys.path.insert(0, '/opt/trn_rl_repo')
import numpy as np
KDBG = bool(os.environ.get('KDBG'))
KSTAGE = os.environ.get('KSTAGE', 'full')

import concourse.bass as bass
from concourse import bacc
import concourse.mybir as mybir
from concourse.tile import TileContext
from concourse.bass_utils import run_bass_kernel_spmd

F32 = mybir.dt.float32
BF16 = mybir.dt.bfloat16
I16 = mybir.dt.int16
ALU = mybir.AluOpType

H, W, C = 96, 128, 128       # per-core shard
NPIX = H * W                 # 12288
NCH = 6                      # h-chunks
CROWS = H // NCH             # 16 rows per chunk
CPIX = CROWS * W             # 2048 px per chunk
ZTOK = NPIX                  # zero token index
SUB = 512


def build_nc():
    nc = bacc.Bacc()
    left_e = nc.declare_dram_parameter("left", [C, H, W], F32, isOutput=False)
    right_e = nc.declare_dram_parameter("right", [C, H, W], F32, isOutput=False)
    flow_e = nc.declare_dram_parameter("flow", [2, H, W], F32, isOutput=False)
    grid_e = nc.declare_dram_parameter("grid", [2, H, W], F32, isOutput=False)
    ones_e = nc.declare_dram_parameter("ones1", [1, 128], F32, isOutput=False)
    selw_e = nc.declare_dram_parameter("selw", [9, 128, 18], F32, isOutput=False)
    out_e = nc.declare_dram_parameter("out", [18, H, W], F32, isOutput=True)

    wscr = nc.dram_tensor("wscratch", [H, 4, W], BF16)
    iscr = nc.dram_tensor("iscratch", [H, 2, W], I16)
    if KDBG:
        dbg_warp = nc.declare_dram_parameter("dbg_warp", [C, H, W], F32, isOutput=True)
        dbg_idx = nc.declare_dram_parameter("dbg_idx", [128, 4, NPIX // 16], I16, isOutput=True)
        dbg_g = nc.declare_dram_parameter("dbg_g", [C, 4, CPIX], F32, isOutput=True)
        dbg_w = nc.declare_dram_parameter("dbg_w", [C, 4, CPIX], F32, isOutput=True)

    with TileContext(nc) as tc:
        with (
            tc.tile_pool(name="per", bufs=1) as per,
            tc.tile_pool(name="rbp", bufs=1) as rbp,
            tc.tile_pool(name="gp", bufs=2) as gp,
            tc.tile_pool(name="wp", bufs=1) as wp,
            tc.tile_pool(name="wfp", bufs=1) as wfp,
            tc.tile_pool(name="lp", bufs=2) as lp,
            tc.tile_pool(name="ltp", bufs=1) as ltp,
            tc.tile_pool(name="tkp", bufs=4) as tkp,
            tc.tile_pool(name="osp", bufs=2) as osp,
            tc.tile_pool(name="cps", bufs=4, space="PSUM") as cps,
            tc.tile_pool(name="wps", bufs=2, space="PSUM") as wps,
        ):
            # ---------- tokens: [w-part, rank h, c] + zero stripe ----------
            # rank r holds [row r-1 | row r] (512B); rows duplicated across slots
            tok = per.tile([128, H + 3, 2, C], BF16, tag="tok", name="tok")
            nc.gpsimd.memset(tok[:, 0, 0, :], 0)
            nc.gpsimd.memset(tok[:, H, 1, :], 0)
            nc.gpsimd.memset(tok[:, H + 1, :, :], 0)
            nc.gpsimd.memset(tok[:, H + 2, :, :], 0)
            for rh in ([] if KSTAGE == 'tok0' else [0, 1, 2, 3]):
                rbh = rbp.tile([C, NPIX // 4], BF16, tag="rbh", name="rbh")
                nc.gpsimd.dma_start(
                    rbh[:],
                    right_e[:, rh * (H // 4):(rh + 1) * (H // 4), :]
                    .rearrange("c h w -> c (h w)"))
                nc.sync.dma_start_transpose(
                    tok[:, 1 + rh * (H // 4):1 + (rh + 1) * (H // 4), 0, :], rbh[:])
                nc.sync.dma_start_transpose(
                    tok[:, rh * (H // 4):(rh + 1) * (H // 4), 1, :], rbh[:])

            # persistent loads (L only needed once taps start)
            L = per.tile([C, NPIX], BF16, tag="L", name="L")
            nc.gpsimd.dma_start(L[:], left_e[:].rearrange("c h w -> c (h w)"))
            ones_tile = per.tile([65, 128], BF16, tag="ones", name="ones")
            nc.gpsimd.dma_start(ones_tile[0:1, :], ones_e[:])
            nc.gpsimd.dma_start(ones_tile[64:65, :], ones_e[:])
            ones_sb = ones_tile[0:1, :]
            ones64 = ones_tile[64:65, :]
            selw_sb = per.tile([128, 9, 18], BF16, tag="selw", name="selw")
            nc.gpsimd.dma_start(
                selw_sb[:], selw_e[:].rearrange("k c j -> c k j"))

            # ---------- params on [96h, 128w], all in one sliced tile -------
            pt = per.tile([H, 25, W], F32, tag="pt", name="pt")
            slot = {}

            def P(tag):
                if tag not in slot:
                    slot[tag] = len(slot)
                    assert len(slot) <= 25, slot
                return pt[:, slot[tag], :]

            def TT(dst, a, b, op):
                nc.vector.tensor_tensor(out=dst, in0=a, in1=b, op=op)

            def TS(dst, a, s1, s2, op0, op1=None):
                kw = {} if op1 is None else {"op1": op1}
                nc.vector.tensor_scalar(out=dst, in0=a, scalar1=s1, scalar2=s2,
                                        op0=op0, **kw)

            nc.sync.dma_start(P("fxin"), flow_e[0])
            nc.sync.dma_start(P("fyin"), flow_e[1])
            nc.sync.dma_start(P("gx"), grid_e[0])
            nc.sync.dma_start(P("gy"), grid_e[1])

            TT(P("x"), P("gx"), P("fxin"), ALU.add)
            TT(P("y"), P("gy"), P("fyin"), ALU.add)
            MAGIC = 12582912.0  # 1.5*2**23 (ulp=1): floor via round(x-0.5)
            TS(P("x0"), P("x"), -0.5, MAGIC, ALU.add, ALU.add)
            TS(P("x0"), P("x0"), -MAGIC, None, ALU.add)
            TS(P("y0"), P("y"), -0.5, MAGIC, ALU.add, ALU.add)
            TS(P("y0"), P("y0"), -MAGIC, None, ALU.add)
            TT(P("fx"), P("x"), P("x0"), ALU.subtract)
            TT(P("fy"), P("y"), P("y0"), ALU.subtract)

            def valid(src, lo, hi, tag):
                TS(P(tag + "a"), src, float(lo), None, ALU.is_ge)
                TS(P("vtmp"), src, float(hi), None, ALU.is_le)
                TT(P(tag + "a"), P(tag + "a"), P("vtmp"), ALU.mult)
                return P(tag + "a")

            vx0 = valid(P("x0"), 0, 127, "vx0")
            vx1 = valid(P("x0"), -1, 126, "vx1")
            vy0 = valid(P("y0"), 0, 95, "vy0")
            vy1 = valid(P("y0"), -1, 94, "vy1")

            s = 1.0 / 64.0
            TS(P("wx0"), P("fx"), -s, s, ALU.mult, ALU.add)
            TS(P("wx1"), P("fx"), s, None, ALU.mult)
            TS(P("wy0"), P("fy"), -1.0, 1.0, ALU.mult, ALU.add)
            TS(P("wy1"), P("fy"), 1.0, None, ALU.mult)

            wmap = per.tile([H, 4, W], F32, tag="wmap", name="wmap")
            TT(wmap[:, 0, :], P("wx0"), P("wy0"), ALU.mult)
            TT(wmap[:, 1, :], P("wx0"), P("wy1"), ALU.mult)
            TT(wmap[:, 2, :], P("wx1"), P("wy0"), ALU.mult)
            TT(wmap[:, 3, :], P("wx1"), P("wy1"), ALU.mult)

            TS(P("xc0"), P("x0"), 0.0, 127.0, ALU.max, ALU.min)
            TS(P("xc1"), P("x0"), 1.0, 0.0, ALU.add, ALU.max)
            TS(P("xc1"), P("xc1"), 127.0, None, ALU.min)
            # pair row base: (y0+1)*128, pair-validity y0 in [-1, 95]
            TS(P("yb0"), P("y0"), 1.0, 128.0, ALU.add, ALU.mult)
            TS(P("pya"), P("y0"), -1.0, None, ALU.is_ge)
            TS(P("vtmp"), P("y0"), 95.0, None, ALU.is_le)
            TT(P("pya"), P("pya"), P("vtmp"), ALU.mult)

            ZP = 97 * 128  # zero-pair token (ranks 97,98 are zero)
            imap = per.tile([H, 2, W], F32, tag="imap", name="imap")
            for n, (xcn, vxn) in enumerate([("xc0", vx0), ("xc1", vx1)]):
                TT(P("bn"), P("yb0"), P(xcn), ALU.add)
                TT(P("vn"), P("pya"), vxn, ALU.mult)
                TS(P("bn"), P("bn"), -float(ZP), None, ALU.add)
                TT(P("bn"), P("bn"), P("vn"), ALU.mult)
                TS(imap[:, n, :], P("bn"), float(ZP), None, ALU.add)

            # cast + micro-permute w -> (q*8 + wh) in one copy
            imap16t = per.tile([H, 2, W], I16, tag="imap16t", name="imap16t")
            nc.vector.tensor_copy(
                imap16t[:].rearrange("h n (q wh) -> h n q wh", q=16),
                imap[:].rearrange("h n (wh q) -> h n q wh", wh=8))

            # weights/indices via DRAM rearrange bounce
            nc.gpsimd.dma_start(wscr[:], wmap[:])
            nc.sync.dma_start(iscr[:], imap16t[:])

            idx_all = per.tile([128, 2, NPIX // 16], I16, tag="idx", name="idx")
            # row p=(ph*16+q) holds, at (n, j), idx of pixel j*16 + q
            # j = h*8 + wh ; pixel = h*128 + wh*16 + q
            # iscr[h, n, q*8 + wh] = idx_n(h, wh*16 + q)
            for n in range(2):
                nc.sync.dma_start(
                    idx_all[0:16, n].rearrange("q (h wh) -> q h wh", wh=8),
                    iscr[:, n].rearrange("h (q wh) -> q h wh", q=16))
            for ph in range(1, 8):
                nc.sync.dma_start(idx_all[ph * 16:(ph + 1) * 16], idx_all[0:16])

            # ---------- per-chunk pipeline ----------
            for ci in range(NCH):
                p0 = ci * CPIX
                h0 = ci * CROWS
                # gathers
                G4 = gp.tile([128, 4, CPIX], BF16, tag="G", name="G%d" % ci)
                for n in range(2 if KSTAGE != 'noga' else 0):
                    nc.gpsimd.dma_gather(
                        out_ap=G4[:, 2 * n:2 * n + 2, :],
                        in_ap=tok[:].rearrange("p h s c -> p (h s c)"),
                        idxs_ap=idx_all[:, n, ci * (CPIX // 16):(ci + 1) * (CPIX // 16)],
                        num_idxs=CPIX, num_idxs_reg=CPIX, elem_size=2 * C,
                        transpose=True, sbuf_tokens_per_rank=128,
                        sbuf_free_dim_per_rank=C * 4, single_packet=False)

                if KSTAGE in ('tok', 'idx', 'noga'):
                    osbg2 = osp.tile([18, CPIX], BF16, tag="osb", name="osbg2%d" % ci)
                    if KSTAGE in ('tok', 'noga'):
                        nc.vector.tensor_copy(
                            osbg2[:], tok[0:18, h0:h0 + CROWS, :]
                            .rearrange("p h c -> p (h c)"))
                    else:
                        nc.vector.tensor_copy(
                            osbg2[:, 0:768],
                            idx_all[0:18, 0, :].bitcast(BF16))
                    nc.gpsimd.dma_start(
                        out_e[:, h0:h0 + CROWS, :].rearrange("j h w -> j (h w)"),
                        osbg2[:])
                    continue
                if KSTAGE == 'gather':
                    osbg = osp.tile([18, CPIX], BF16, tag="osb", name="osbg%d" % ci)
                    nc.vector.tensor_copy(osbg[:], G4[0:18, 0, :])
                    nc.gpsimd.dma_start(
                        out_e[:, h0:h0 + CROWS, :].rearrange("j h w -> j (h w)"),
                        osbg[:])
                    continue
                # weight maps for this chunk: [1, 4*CPIX] bf16 <- DRAM
                wfc = wfp.tile([65, 2, CPIX], BF16, tag="wfc", name="wfc%d" % ci)
                for n in range(4):
                    nc.sync.dma_start(
                        wfc[(n % 2) * 64:(n % 2) * 64 + 1, n // 2]
                        .rearrange("o (h w) -> o h w", w=W),
                        wscr[h0:h0 + CROWS, n])
                W4 = wp.tile([128, 4, CPIX], BF16, tag="W", name="W%d" % ci)
                for n in range(4):
                    for hf in range(2):
                        ps = wps.tile([128, CPIX // 2], F32, tag="wps",
                                      name="wps")
                        for sb in range(2):
                            o0 = hf * (CPIX // 2) + sb * SUB
                            nc.tensor.matmul(
                                ps[:, sb * SUB:(sb + 1) * SUB],
                                ones_sb if n % 2 == 0 else ones64,
                                wfc[(n % 2) * 64:(n % 2) * 64 + 1, n // 2,
                                    o0:o0 + SUB])
                        nc.scalar.copy(
                            W4[:, n, hf * (CPIX // 2):(hf + 1) * (CPIX // 2)],
                            ps[:])

                # lerp -> padded tiles
                pe = lp.tile([128, CROWS, 136], BF16, tag="pe", name="pe%d" % ci)
                po = lp.tile([128, CROWS, 138], BF16, tag="po", name="po%d" % ci)
                t0 = ltp.tile([128, CPIX], BF16, tag="lt0", name="lt0")
                t1 = ltp.tile([128, CPIX], BF16, tag="lt1", name="lt1")
                TT(t0[:], G4[:, 0, :], W4[:, 0, :], ALU.mult)
                TT(t1[:], G4[:, 1, :], W4[:, 1, :], ALU.mult)
                TT(t0[:], t0[:], t1[:], ALU.add)
                TT(t1[:], G4[:, 2, :], W4[:, 2, :], ALU.mult)
                TT(t0[:], t0[:], t1[:], ALU.add)
                TT(t1[:], G4[:, 3, :], W4[:, 3, :], ALU.mult)
                nc.vector.tensor_tensor(
                    out=pe[:, :, 4:132],
                    in0=t0[:].rearrange("c (h w) -> c h w", w=W),
                    in1=t1[:].rearrange("c (h w) -> c h w", w=W), op=ALU.add)
                for j in range(4):
                    nc.vector.tensor_copy(pe[:, :, j:j + 1], pe[:, :, 4:5])
                    nc.vector.tensor_copy(pe[:, :, 132 + j:133 + j],
                                          pe[:, :, 131:132])
                nc.sync.dma_start(po[:, :, 1:137], pe[:, :, 0:136])
                if KDBG:
                    nc.gpsimd.dma_start(
                        dbg_warp[:, h0:h0 + CROWS, :], pe[:, :, 4:132])
                    if ci == 0:
                        nc.gpsimd.dma_start(dbg_idx[:], idx_all[:])
                        nc.gpsimd.dma_start(dbg_g[:], G4[:])
                        nc.gpsimd.dma_start(dbg_w[:], W4[:])

                # taps + selector matmuls
                osb = osp.tile([18, CPIX], F32, tag="osb", name="osb%d" % ci)
                if KSTAGE == 'warp':
                    nc.vector.tensor_copy(
                        osb[:].rearrange("c (h w) -> c h w", w=W),
                        pe[0:18, :, 4:132])
                    nc.gpsimd.dma_start(
                        out_e[:, h0:h0 + CROWS, :].rearrange("j h w -> j (h w)"),
                        osb[:])
                    continue
                for tc2 in range(2):
                    q0 = tc2 * (CPIX // 2)
                    lsl = L[:, p0 + q0: p0 + q0 + CPIX // 2]
                    pss = [cps.tile([128, SUB], F32, tag="cpsum",
                                    name="cps%d_%d_%d" % (ci, tc2, i))
                           for i in range(2)]
                    korder = [0, 2, 4, 6, 8, 1, 3, 5, 7]
                    for ki, k in enumerate(korder):
                        tk = tkp.tile([128, CPIX // 2], BF16, tag="tk",
                                      name="tk")
                        if k % 2 == 0:
                            rw = pe[:, tc2 * 8:(tc2 + 1) * 8, k:k + 128]
                        else:
                            rw = po[:, tc2 * 8:(tc2 + 1) * 8, k + 1:k + 129]
                        nc.vector.tensor_tensor(
                            out=tk[:].rearrange("c (h w) -> c h w", w=W),
                            in0=lsl.rearrange("c (h w) -> c h w", w=W),
                            in1=rw, op=ALU.mult)
                        for sb in range(2):
                            nc.tensor.matmul(
                                pss[sb][0:18, :], selw_sb[:, k, :],
                                tk[:, sb * SUB:(sb + 1) * SUB],
                                start=(ki == 0), stop=(ki == 8))
                    for sb in range(2):
                        nc.scalar.copy(
                            osb[:, q0 + sb * SUB:q0 + (sb + 1) * SUB],
                            pss[sb][0:18, :])
                nc.sync.dma_start(
                    out_e[:, h0:h0 + CROWS, :].rearrange("j h w -> j (h w)"),
                    osb[:])
    nc.compile()
    return nc


_NC_CACHE = {}


def _get_nc():
    if "nc" not in _NC_CACHE:
        _NC_CACHE["nc"] = build_nc()
    return _NC_CACHE["nc"]


def _consts():
    yy, xx = np.meshgrid(np.arange(H, dtype=np.float32),
                         np.arange(W, dtype=np.float32), indexing="ij")
    grid = np.stack([xx, yy], axis=0).astype(np.float32)  # [2, H, W] (x, y)
    ones1 = np.ones((1, 128), np.float32)
    selw = np.zeros((9, 128, 18), np.float32)
    for k in range(9):
        for c in range(128):
            g = c // 64
            selw[k, c, g * 9 + k] = 1.0
    return grid, ones1, selw


def _in_maps(left_features, right_features, flow):
    grid, ones1, selw = _consts()
    in_maps = []
    for core in range(8):
        b, half = core // 2, core % 2
        in_maps.append({
            "left": np.ascontiguousarray(
                left_features[b, half * 128:(half + 1) * 128]).astype(np.float32),
            "right": np.ascontiguousarray(
                right_features[b, half * 128:(half + 1) * 128]).astype(np.float32),
            "flow": np.ascontiguousarray(flow[b]).astype(np.float32),
            "grid": grid, "ones1": ones1, "selw": selw,
        })
    return in_maps


def kernel(left_features, right_features, flow):
    nc = _get_nc()
    res = run_bass_kernel_spmd(nc, _in_maps(left_features, right_features, flow),
                               core_ids=list(range(8)))
    out = np.zeros((4, 36, H, W), np.float32)
    for core in range(8):
        b, half = core // 2, core % 2
        out[b, half * 18:(half + 1) * 18] = res.results[core]["out"]
    return out
